# revision 4
# baseline (speedup 1.0000x reference)
"""BackgroundNoiseLayer kernel for 8 trn2 NeuronCores.

Math: out[0, t, n] = sum_k W[n, k] * rest[t, k], where W [60000, 100] is
scatter-added from COO (v1 block rows 0..49999, lm block rows 50000..59999)
and the output feature axis is the concat of the two blocks.

Strategy (per sharding hint): densify the tiny sparse matrix host-side
(240k nnz -> dense W, ~0.002% of the matmul FLOPs), shard the post-synaptic
feature axis across the 8 cores (7500 features each), and run a dense
[1000,100] @ [100,7500] matmul per core. rest is tiny and replicated. Each
core writes its own contiguous output slice; concat on host is the no-op
gather.

Device-side pipeline (from profiling):
- Correctness gate is rel_err < 2e-2; bf16 weights (one plane, ~1e-3 rel)
  and an fp16 output stream (fp32 PSUM -> fp16 on the PSUM-copy, host
  upconverts) halve the dominant HBM write stream vs f32 and cut PE time
  3x vs the fp32-exact 3-plane variant.
- Three rate-matched stations: PE (bf16 matmuls, N=512 per PSUM bank),
  PSUM evacuation (DVE+ACT alternating; fp32 PSUM reads are capped at
  1 elem/cycle/partition), out-DMA (~400 GB/s aggregate on the sync
  HWDGE ring). Copies run on [128,1024] double-bank PSUM tiles so the
  fixed per-instruction overhead (120/172 cycles) amortizes over 2 banks.
- Last double-chunk is 332 real columns (7500 = 7*1024 + 332): no padded
  compute/copy/DMA on the feature axis.
- sync ring carries only the output stream (plus the first row-block's
  inputs to cut time-to-first-matmul); all other input loads ride the
  gpsimd SWDGE ring. Row-blocks 0-1 stream per-double-chunk pieces so the
  write stream starts as soon as the first chunk is staged.
"""

import os

import numpy as np

B, T = 1, 1000
NBKG = 100
NV1, NLM = 50000, 10000
NPOST = NV1 + NLM          # 60000
NCORES = 8
SHARD = NPOST // NCORES    # 7500 real features per core

KP = 112                   # padded contraction dim (zeros in rows 100..111)
ROWS = 1024                # padded time rows (zeros in 1000..1023)
TBLK = 128                 # rows per block = full partition set
NT = ROWS // TBLK          # 8
MMN = 512                  # matmul free dim cap = one fp32 PSUM bank
DCW = 1024                 # double-chunk width (2 PSUM banks)
# double-chunks per row block: 7 x 1024 + 1 x 332 = 7500
DCHUNKS = [(i * DCW, DCW) for i in range(7)] + [(7 * DCW, SHARD - 7 * DCW)]
RAMP = 2                   # row blocks that stream per-double-chunk pieces

_compiled = None


def _build_module():
    import concourse.bacc as bacc
    import concourse.mybir as mybir
    import concourse.tile as tile

    f32 = mybir.dt.float32
    f16 = mybir.dt.float16
    bf16 = mybir.dt.bfloat16
    nc = bacc.Bacc("TRN2", target_bir_lowering=False, debug=False)
    restT = nc.dram_tensor("restT", [KP, ROWS], bf16, kind="ExternalInput")
    wT = nc.dram_tensor("wT", [KP, SHARD], bf16, kind="ExternalInput")
    out = nc.dram_tensor("out", [ROWS, SHARD], f16, kind="ExternalOutput")

    with tile.TileContext(nc) as tc:
        with (
            tc.tile_pool(name="inp", bufs=1) as inp,
            tc.tile_pool(name="stage", bufs=3) as stagep,
            tc.tile_pool(name="psum", bufs=4, space="PSUM") as psump,
        ):
            # first two row blocks' lhsT in a small early tile on the fast
            # HWDGE ring; the rest in one big tile on the SWDGE ring
            rest0 = inp.tile([KP, 2 * TBLK], bf16, tag="rest0")
            nc.sync.dma_start(rest0[:], restT[:, :2 * TBLK])
            rest1 = inp.tile([KP, ROWS - 2 * TBLK], bf16, tag="rest1")
            nc.gpsimd.dma_start(rest1[:], restT[:, 2 * TBLK:])

            # one tile per w double-chunk so the first matmuls gate only on
            # their own slice; w0 on sync (fast first-byte, before any out
            # DMA), w1 on scalar (before any copies), rest on gpsimd
            w_sb = []
            for j, (off, w) in enumerate(DCHUNKS):
                wt = inp.tile([KP, w], bf16, tag=f"w{j}", name=f"w{j}")
                ring = nc.sync if j == 0 else (
                    nc.scalar if j == 1 else nc.gpsimd)
                ring.dma_start(wt[:], wT[:, off:off + w])
                w_sb.append(wt)

            # copy split: vector (slower per elem) takes 3x1024 + the 332
            # tail, scalar takes 4x1024 -> per-block walls ~4.1 vs ~3.9 us
            vector_chunks = {0, 2, 4, 7}
            for tb in range(NT):
                ramp = tb < RAMP
                if not ramp:
                    stage = stagep.tile([TBLK, SHARD], f16, tag="stage",
                                        name=f"stage{tb}", bufs=3)
                if tb < 2:
                    lhsT = rest0[:, tb * TBLK:(tb + 1) * TBLK]
                else:
                    lhsT = rest1[:, (tb - 2) * TBLK:(tb - 1) * TBLK]
                for j, (off, w) in enumerate(DCHUNKS):
                    if ramp:
                        stage = stagep.tile([TBLK, w], f16, tag=f"r{tb}_{j}",
                                            name=f"r{tb}_{j}", bufs=1)
                    soff = 0 if ramp else off
                    ps = psump.tile([TBLK, DCW], f32, tag="ps")
                    for m in range((w + MMN - 1) // MMN):
                        n0 = m * MMN
                        n1 = min(w, n0 + MMN)
                        mm = nc.tensor.matmul(
                            ps[:, n0:n1],
                            lhsT,
                            w_sb[j][:, n0:n1],
                            start=True,
                            stop=True,
                        )
                        # the stationary operand (rest row-block) is the
                        # same for all 15 matmuls of a block: only the
                        # first needs the LDWEIGHTS (saves ~1.6us/block
                        # of PE pipe time)
                        if j > 0 or m > 0:
                            mm.ins.ldweights = False
                    copy = (nc.vector.tensor_copy if j in vector_chunks
                            else nc.scalar.copy)
                    copy(stage[:, soff:soff + w], ps[:, :w])
                    if ramp:
                        nc.sync.dma_start(
                            out[tb * TBLK:(tb + 1) * TBLK, off:off + w],
                            stage[:],
                        )
                if not ramp:
                    nc.sync.dma_start(
                        out[tb * TBLK:(tb + 1) * TBLK, :], stage[:]
                    )

    nc.compile()
    return nc


def _densify(v1_weights, v1_rows, v1_cols, lm_weights, lm_rows, lm_cols):
    rows = np.concatenate([
        np.asarray(v1_rows).astype(np.int64),
        np.asarray(lm_rows).astype(np.int64) + NV1,
    ])
    cols = np.concatenate([
        np.asarray(v1_cols).astype(np.int64),
        np.asarray(lm_cols).astype(np.int64),
    ])
    w = np.concatenate([
        np.asarray(v1_weights, dtype=np.float32),
        np.asarray(lm_weights, dtype=np.float32),
    ])
    W = np.bincount(rows * NBKG + cols, weights=w, minlength=NPOST * NBKG)
    return W.astype(np.float32).reshape(NPOST, NBKG)


def kernel(rest, v1_weights, v1_rows, v1_cols, lm_weights, lm_rows, lm_cols):
    import ml_dtypes

    from concourse.bass_utils import run_bass_kernel_spmd

    bf16 = ml_dtypes.bfloat16

    global _compiled
    if _compiled is None:
        _compiled = _build_module()

    W = _densify(v1_weights, v1_rows, v1_cols, lm_weights, lm_rows, lm_cols)
    w_hi = W.astype(bf16)

    rest32 = np.asarray(rest, np.float32)
    rest_b = rest32.astype(bf16)

    restT = np.zeros((KP, ROWS), bf16)
    restT[:NBKG, :B * T] = rest_b.T

    in_maps = []
    for c in range(NCORES):
        wpad = np.zeros((KP, SHARD), bf16)
        wpad[:NBKG, :] = w_hi[c * SHARD:(c + 1) * SHARD].T
        in_maps.append({"restT": restT, "wT": wpad})

    trace = bool(int(os.environ.get("KERNEL_TRACE", "0")))
    if trace:
        _install_ntff_shim()
    res = run_bass_kernel_spmd(
        _compiled, in_maps, core_ids=list(range(NCORES)), trace=trace
    )
    kernel.last_results = res
    full = np.concatenate(
        [res.results[c]["out"][:B * T, :].astype(np.float32)
         for c in range(NCORES)],
        axis=1,
    )
    return full.reshape(B, T, NPOST)


def _install_ntff_shim():
    """The agent image's antenv lacks axon_hooks; register the NTFF profile
    hook by dlopening libaxon_pjrt.so directly (same path trn_boot uses)."""
    import sys
    import types

    if "antenv.axon_hooks" in sys.modules:
        return
    try:
        from trn_agent_boot.trn_boot import _ntff_profile_via_ctypes

        hook = _ntff_profile_via_ctypes("/opt/axon/libaxon_pjrt.so")
    except Exception:
        hook = None
    mod = types.ModuleType("antenv.axon_hooks")
    mod.get_axon_ntff_profile_hook = lambda: hook
    mod.set_axon_ntff_profile_hook = lambda h: None
    sys.modules["antenv.axon_hooks"] = mod
